# revision 1
# baseline (speedup 1.0000x reference)
"""Trainium2 Bass kernel for 3-layer EGAT message passing (nn_COUNTYOD).

Strategy (8 cores, edge parallelism by dst range):
  - Host: sort edges by dst; device d owns dst nodes [d*6272, (d+1)*6272);
    edges grouped into 49 blocks of 128 dst nodes, padded to T tiles of
    128 edges per block (T = data max, ~17).
  - Tables cat[n] = [nh@Wni + b | nh@Wnj | nh@Wns]  (N x 288) computed as
    a per-device slice then AllGather'd (the only collective per layer).
  - Edge pass 1: f_pre = ni[src] + nj[dst] + ef@Wfij (indirect-DMA row
    gathers from cat + PE matmul); e-logits via DVE mul+grouped reduce;
    relu(f_pre) stored (transposed by PE) as next-layer edge features.
  - Softmax with per-device per-head GLOBAL max (exact: segments are
    device-local; any per-head constant cancels).
  - Edge pass 2: msg = ex * ns[src]; scatter-add via one-hot matmul into
    PSUM per 128-node block; s row appended (col 96:99); h = acc/s.
"""

import os
import sys
import numpy as np

for _p in ("/opt/trn_rl_repo",):
    if _p not in sys.path:
        sys.path.insert(0, _p)

import concourse.bass as bass  # noqa: E402
import concourse.bacc as bacc  # noqa: E402
import concourse.mybir as mybir  # noqa: E402
import concourse.tile as tile  # noqa: E402
from concourse.bass import IndirectOffsetOnAxis  # noqa: E402
from concourse.masks import make_identity  # noqa: E402

F32 = mybir.dt.float32
I32 = mybir.dt.int32
AF = mybir.ActivationFunctionType
ALU = mybir.AluOpType

P = 128
FD = 96           # H*HE
CAT = 288
H = 3
HE = 32
EPS = 1e-20


class Cfg:
    def __init__(self, ndev, nblk, T, odf, n_real, g1=4):
        self.ndev = ndev
        self.nblk = nblk              # blocks (of 128 dst nodes) per device
        self.T = T                    # edge tiles per block
        self.odf = odf                # layer-1 edge feature dim
        self.n_real = n_real
        self.g1 = g1                  # pass-1 chunk size (tiles)
        self.R = nblk * P             # nodes per device
        self.npad = ndev * self.R
        self.ttot = nblk * T          # tiles per device
        self.epd = self.ttot * P      # padded edges per device


def host_prep(inputs, cfg):
    """Sort/pad/shard on host. Returns per-core input maps."""
    src = np.asarray(inputs["src"]).astype(np.int64)
    dst = np.asarray(inputs["dst"]).astype(np.int64)

    order = np.argsort(dst, kind="stable")
    ssrc, sdst = src[order], dst[order]
    ef0 = np.asarray(inputs["countyodfeats"]).astype(np.float32)[order]

    nb_all = cfg.npad // P
    blk = sdst // P
    blkcnt = np.bincount(blk, minlength=nb_all)
    starts = np.zeros(nb_all + 1, np.int64)
    np.cumsum(blkcnt, out=starts[1:])

    T, TP = cfg.T, cfg.T * P
    percore = []
    for d in range(cfg.ndev):
        sidx = np.zeros((cfg.nblk, TP), np.int64)
        didx = np.zeros((cfg.nblk, TP), np.int64)
        dloc = np.full((cfg.nblk, TP), -1.0, np.float32)
        efp = np.zeros((cfg.nblk, TP, cfg.odf), np.float32)
        for b in range(cfg.nblk):
            g = d * cfg.nblk + b
            s0, s1 = starts[g], starts[g + 1]
            n = s1 - s0
            assert n <= TP
            sidx[b, :n] = ssrc[s0:s1]
            didx[b, :n] = sdst[s0:s1]
            dloc[b, :n] = (sdst[s0:s1] - g * P).astype(np.float32)
            efp[b, :n] = ef0[s0:s1]
        sidx = sidx.reshape(-1, P).T  # [128, ttot]
        didx = didx.reshape(-1, P).T
        dloc = dloc.reshape(-1, P).T
        percore.append(
            dict(
                srcidx=np.ascontiguousarray(sidx.astype(np.int32)),
                dstidx=np.ascontiguousarray(didx.astype(np.int32)),
                dstloc=np.ascontiguousarray(dloc.astype(np.float32)),
                ef0T=np.ascontiguousarray(efp.reshape(-1, cfg.odf).T),
            )
        )

    nh0 = np.asarray(inputs["ndata_h"]).astype(np.float32)
    nh0p = np.zeros((cfg.npad, FD), np.float32)
    nh0p[: cfg.n_real] = nh0[: cfg.n_real]
    for d in range(cfg.ndev):
        sl = nh0p[d * cfg.R : (d + 1) * cfg.R]
        percore[d]["nh0T"] = np.ascontiguousarray(
            np.concatenate([sl.T, np.ones((1, cfg.R), np.float32)], axis=0)
        )

    def wcat_ext(Wni, Wnj, Wns, bias):
        w = np.concatenate([Wni, Wnj, Wns], axis=1)
        b = np.concatenate([bias, np.zeros(2 * FD, np.float32)])[None, :]
        return np.ascontiguousarray(np.concatenate([w, b], axis=0).astype(np.float32))

    g = lambda k: np.asarray(inputs[k]).astype(np.float32)
    weights = dict(
        wcat0=wcat_ext(g("Wni0"), g("Wnj0"), g("Wns0"), g("bias0")),
        wcat1=wcat_ext(g("Wni")[0], g("Wnj")[0], g("Wns")[0], g("bias")[0]),
        wcat2=wcat_ext(g("Wni")[1], g("Wnj")[1], g("Wns")[1], g("bias")[1]),
        wfij0=np.ascontiguousarray(g("Wfij0")),
        wfij1=np.ascontiguousarray(g("Wfij")[0]),
        wfij2=np.ascontiguousarray(g("Wfij")[1]),
        attn0=np.ascontiguousarray(np.repeat(g("attn0").reshape(1, FD), 128, 0)),
        attn1=np.ascontiguousarray(np.repeat(g("attn").reshape(2, FD)[0:1], 128, 0)),
        attn2=np.ascontiguousarray(np.repeat(g("attn").reshape(2, FD)[1:2], 128, 0)),
    )
    for d in range(cfg.ndev):
        percore[d].update(weights)
    return percore


def build_program(cfg, debug=False):
    nc = bacc.Bacc("TRN2", target_bir_lowering=False, debug=False)
    c = cfg
    Ttot, EPD = c.ttot, c.epd

    # ---- I/O ----
    pr = {}
    pr["nh0T"] = nc.declare_dram_parameter("nh0T", [FD + 1, c.R], F32, isOutput=False)
    pr["ef0T"] = nc.declare_dram_parameter("ef0T", [c.odf, EPD], F32, isOutput=False)
    pr["srcidx"] = nc.declare_dram_parameter("srcidx", [P, Ttot], I32, isOutput=False)
    pr["dstidx"] = nc.declare_dram_parameter("dstidx", [P, Ttot], I32, isOutput=False)
    pr["dstloc"] = nc.declare_dram_parameter("dstloc", [P, Ttot], F32, isOutput=False)
    for i in range(3):
        pr[f"wcat{i}"] = nc.declare_dram_parameter(f"wcat{i}", [FD + 1, CAT], F32, isOutput=False)
        pr[f"attn{i}"] = nc.declare_dram_parameter(f"attn{i}", [P, FD], F32, isOutput=False)
    pr["wfij0"] = nc.declare_dram_parameter("wfij0", [c.odf, FD], F32, isOutput=False)
    pr["wfij1"] = nc.declare_dram_parameter("wfij1", [FD, FD], F32, isOutput=False)
    pr["wfij2"] = nc.declare_dram_parameter("wfij2", [FD, FD], F32, isOutput=False)
    out3 = nc.declare_dram_parameter("out3", [c.R, FD], F32, isOutput=True)
    dbg = {}
    if debug:
        dbg["d_cat"] = nc.declare_dram_parameter("d_cat", [c.npad, CAT], F32, isOutput=True)
        dbg["d_eraw"] = nc.declare_dram_parameter("d_eraw", [P, H * c.ttot], F32, isOutput=True)
        dbg["d_ex"] = nc.declare_dram_parameter("d_ex", [P, H * c.ttot], F32, isOutput=True)
        dbg["d_ef"] = nc.declare_dram_parameter("d_ef", [FD, c.epd], F32, isOutput=True)
        dbg["d_nh"] = nc.declare_dram_parameter("d_nh", [FD + 1, c.R], F32, isOutput=True)
        dbg["d_ni"] = nc.declare_dram_parameter("d_ni", [P, c.ttot * FD], F32, isOutput=True)

    # ---- internal DRAM ----
    efA = nc.dram_tensor("efA", [FD, EPD], F32)
    efB = nc.dram_tensor("efB", [FD, EPD], F32)
    catL = nc.dram_tensor("catL", [c.R, CAT], F32)
    catG = nc.dram_tensor("catG", [c.npad, CAT], F32, addr_space="Shared")

    rg = [list(range(c.ndev))]

    with tile.TileContext(nc) as tc:
        with tc.tile_pool(name="persist", bufs=1) as pp:
            # constants
            ident = pp.tile([P, P], F32, tag="ident")
            make_identity(nc, ident[:])
            iota_i = pp.tile([P, P], I32, tag="iota_i")
            nc.gpsimd.iota(iota_i[:], pattern=[[1, P]], base=0, channel_multiplier=0)
            iota_f = pp.tile([P, P], F32, tag="iota_f")
            nc.vector.tensor_copy(out=iota_f[:], in_=iota_i[:])

            srcidx = pp.tile([P, Ttot], I32, tag="srcidx")
            dstidx = pp.tile([P, Ttot], I32, tag="dstidx")
            dstloc = pp.tile([P, Ttot], F32, tag="dstloc")
            nc.sync.dma_start(out=srcidx[:], in_=pr["srcidx"][:])
            nc.sync.dma_start(out=dstidx[:], in_=pr["dstidx"][:])
            nc.sync.dma_start(out=dstloc[:], in_=pr["dstloc"][:])

            wfij_sb = []
            wcat_sb = []
            abc_sb = []
            if debug:
                nc.sync.dma_start(out=dbg["d_cat"][:], in_=catG[:])
            for l in range(3):
                cdim = c.odf if l == 0 else FD
                wf = pp.tile([cdim, FD], F32, tag=f"wfij{l}")
                nc.sync.dma_start(out=wf[:], in_=pr[f"wfij{l}"][:])
                wfij_sb.append(wf)
                wc = pp.tile([FD + 1, CAT], F32, tag=f"wcat{l}")
                nc.sync.dma_start(out=wc[:], in_=pr[f"wcat{l}"][:])
                wcat_sb.append(wc)
                abc = pp.tile([P, FD], F32, tag=f"abc{l}")
                nc.sync.dma_start(out=abc[:], in_=pr[f"attn{l}"][:])
                abc_sb.append(abc)

            nh_slice = pp.tile([FD + 1, c.R], F32, tag="nh_slice")
            nc.sync.dma_start(out=nh_slice[:], in_=pr["nh0T"][:])
            e_buf = pp.tile([P, H * Ttot], F32, tag="e_buf")

            def e_ap(offset, ap):
                return bass.AP(e_buf[:].tensor, e_buf[:].offset + offset, ap)

            ebs = e_buf[:].ap[0][0]  # partition stride of e_buf

            def tables(nh_sb, wc):
                with (
                    tc.tile_pool(name="tabw", bufs=2) as wp,
                    tc.tile_pool(name="tabp", bufs=2, space="PSUM") as qp,
                ):
                    for nb in range(c.nblk):
                        pt = qp.tile([P, CAT], F32, tag="ptab")
                        nc.tensor.matmul(
                            out=pt[:],
                            lhsT=nh_sb[:, nb * P : (nb + 1) * P],
                            rhs=wc[:],
                            start=True,
                            stop=True,
                        )
                        cs = wp.tile([P, CAT], F32, tag="catsb")
                        nc.scalar.activation(out=cs[:], in_=pt[:], func=AF.Copy)
                        nc.sync.dma_start(
                            out=catL[nb * P : (nb + 1) * P, :], in_=cs[:]
                        )
                nc.gpsimd.collective_compute(
                    "AllGather",
                    ALU.bypass,
                    replica_groups=rg,
                    ins=[catL[:]],
                    outs=[catG[:]],
                )

            # layer 0 tables from input node features
            tables(nh_slice, wcat_sb[0])

            if debug:
                nc.sync.dma_start(out=dbg["d_cat"][:], in_=catG[:])
            for l in range(3):
                cdim = c.odf if l == 0 else FD
                ef_src = pr["ef0T"] if l == 0 else (efA if l == 1 else efB)
                ef_dst = efA if l == 0 else efB
                store_f = l < 2
                wf = wfij_sb[l]
                abc = abc_sb[l]

                # ---------------- pass 1: f_pre, e-logits ----------------
                with (
                    tc.tile_pool(name="p1", bufs=3) as w1,
                    tc.tile_pool(name="p1p", bufs=2, space="PSUM") as q1,
                ):
                    for c0 in range(0, Ttot, c.g1):
                        g = min(c.g1, Ttot - c0)
                        W = g * FD
                        ni_g = w1.tile([P, W], F32, tag="nig")
                        nj_g = w1.tile([P, W], F32, tag="njg")
                        for j in range(g):
                            nc.gpsimd.indirect_dma_start(
                                out=ni_g[:, j * FD : (j + 1) * FD],
                                out_offset=None,
                                in_=catG[:, :],
                                in_offset=IndirectOffsetOnAxis(
                                    ap=srcidx[:, c0 + j : c0 + j + 1], axis=0
                                ),
                                element_offset=0,
                            )
                            nc.gpsimd.indirect_dma_start(
                                out=nj_g[:, j * FD : (j + 1) * FD],
                                out_offset=None,
                                in_=catG[:, :],
                                in_offset=IndirectOffsetOnAxis(
                                    ap=dstidx[:, c0 + j : c0 + j + 1], axis=0
                                ),
                                element_offset=FD,
                            )
                        if debug and l == 0:
                            nc.sync.dma_start(
                                out=dbg["d_ni"][:, c0 * FD : (c0 + g) * FD],
                                in_=ni_g[:],
                            )
                        efc = w1.tile([cdim, g * P], F32, tag="efc")
                        nc.sync.dma_start(
                            out=efc[:],
                            in_=ef_src[:, c0 * P : (c0 + g) * P],
                        )
                        fp = q1.tile([P, W], F32, tag="fps")
                        for j in range(g):
                            nc.tensor.matmul(
                                out=fp[:, j * FD : (j + 1) * FD],
                                lhsT=efc[:, j * P : (j + 1) * P],
                                rhs=wf[:],
                                start=True,
                                stop=True,
                            )
                        t2 = w1.tile([P, W], F32, tag="t2")
                        nc.vector.tensor_add(out=t2[:], in0=fp[:], in1=ni_g[:])
                        nc.vector.tensor_add(out=t2[:], in0=t2[:], in1=nj_g[:])
                        # leaky = 0.99*relu(t2) + 0.01*t2
                        fs99 = w1.tile([P, W], F32, tag="fs99")
                        nc.scalar.activation(
                            out=fs99[:], in_=t2[:], func=AF.Relu, scale=0.99
                        )
                        fl = w1.tile([P, W], F32, tag="fl")
                        nc.vector.tensor_scalar_mul(
                            out=fl[:], in0=t2[:], scalar1=0.01
                        )
                        nc.vector.tensor_add(
                            out=fl[:], in0=fl[:], in1=fs99[:]
                        )
                        # e logits: mul by attn row, grouped reduce
                        em = w1.tile([P, W], F32, tag="em")
                        a_view = bass.AP(
                            abc[:].tensor, abc[:].offset,
                            [abc[:].ap[0], [0, g], [1, FD]],
                        )
                        flv = bass.AP(
                            fl[:].tensor, fl[:].offset,
                            [fl[:].ap[0], [FD, g], [1, FD]],
                        )
                        nc.vector.tensor_tensor(
                            out=em[:].rearrange("p (t f) -> p t f", t=g),
                            in0=flv,
                            in1=a_view,
                            op=ALU.mult,
                        )
                        emv = bass.AP(
                            em[:].tensor, em[:].offset,
                            [em[:].ap[0], [FD, g], [HE, H], [1, HE]],
                        )
                        eo = e_ap(c0, [[ebs, P], [1, g], [Ttot, H]])
                        nc.vector.tensor_reduce(
                            out=eo, in_=emv, axis=mybir.AxisListType.X, op=ALU.add
                        )
                        if store_f:
                            fs = w1.tile([P, W], F32, tag="fs")
                            nc.scalar.activation(
                                out=fs[:], in_=t2[:], func=AF.Relu
                            )
                            pT = q1.tile([FD, g * P], F32, tag="pT")
                            for j in range(g):
                                nc.tensor.transpose(
                                    out=pT[:, j * P : (j + 1) * P],
                                    in_=fs[:, j * FD : (j + 1) * FD],
                                    identity=ident[:],
                                )
                            ftT = w1.tile([FD, g * P], F32, tag="ftT")
                            nc.vector.tensor_copy(
                                out=ftT[:], in_=pT[:]
                            )
                            nc.sync.dma_start(
                                out=ef_dst[:, c0 * P : (c0 + g) * P],
                                in_=ftT[:],
                            )

                # ---------------- softmax: ex = exp(min(e, 60)) ----------------
                if debug and l == 0:
                    nc.sync.dma_start(out=dbg["d_eraw"][:], in_=e_buf[:])
                nc.vector.tensor_scalar(
                    out=e_buf[:], in0=e_buf[:], scalar1=60.0, scalar2=None,
                    op0=ALU.min,
                )
                nc.scalar.activation(out=e_buf[:], in_=e_buf[:], func=AF.Exp)
                if debug and l == 0:
                    nc.sync.dma_start(out=dbg["d_ex"][:], in_=e_buf[:])
                    nc.sync.dma_start(out=dbg["d_ef"][:], in_=efA[:])

                # ---------------- pass 2: scatter ----------------
                with (
                    tc.tile_pool(name="p2", bufs=3) as w2,
                    tc.tile_pool(name="p2p", bufs=2, space="PSUM") as q2,
                    tc.tile_pool(name="p2t", bufs=2, space="PSUM") as q2t,
                ):
                    for b in range(c.nblk):
                        t0 = b * c.T
                        TW = c.T * FD
                        ns_g = w2.tile([P, TW], F32, tag="nsg")
                        for j in range(c.T):
                            nc.gpsimd.indirect_dma_start(
                                out=ns_g[:, j * FD : (j + 1) * FD],
                                out_offset=None,
                                in_=catG[:, :],
                                in_offset=IndirectOffsetOnAxis(
                                    ap=srcidx[:, t0 + j : t0 + j + 1], axis=0
                                ),
                                element_offset=2 * FD,
                            )
                        msg = w2.tile([P, TW], F32, tag="msg")
                        nsv = bass.AP(
                            ns_g[:].tensor, ns_g[:].offset,
                            [ns_g[:].ap[0], [FD, c.T], [HE, H], [1, HE]],
                        )
                        exv = e_ap(t0, [[ebs, P], [1, c.T], [Ttot, H], [0, HE]])
                        nc.vector.tensor_tensor(
                            out=msg[:].rearrange(
                                "p (t h d) -> p t h d", t=c.T, h=H
                            ),
                            in0=nsv,
                            in1=exv,
                            op=ALU.mult,
                        )
                        oh = w2.tile([P, c.T * P], F32, tag="oh")
                        iov = bass.AP(
                            iota_f[:].tensor, iota_f[:].offset,
                            [iota_f[:].ap[0], [0, c.T], [1, P]],
                        )
                        dlv = bass.AP(
                            dstloc[:].tensor, dstloc[:].offset + t0,
                            [dstloc[:].ap[0], [1, c.T], [0, P]],
                        )
                        nc.vector.tensor_tensor(
                            out=oh[:].rearrange("p (t v) -> p t v", t=c.T),
                            in0=iov,
                            in1=dlv,
                            op=ALU.is_equal,
                        )
                        ps = q2.tile([P, FD], F32, tag="ps")
                        ps2 = q2.tile([P, H], F32, tag="ps2")
                        for t in range(c.T):
                            oht = oh[:, t * P : (t + 1) * P]
                            nc.tensor.matmul(
                                out=ps[:],
                                lhsT=oht,
                                rhs=msg[:, t * FD : (t + 1) * FD],
                                start=(t == 0),
                                stop=(t == c.T - 1),
                                skip_group_check=True,
                            )
                            exr = e_ap(t0 + t, [[ebs, P], [Ttot, H]])
                            nc.tensor.matmul(
                                out=ps2[:],
                                lhsT=oht,
                                rhs=exr,
                                start=(t == 0),
                                stop=(t == c.T - 1),
                                skip_group_check=True,
                            )
                        sp = w2.tile([P, H], F32, tag="sp")
                        nc.vector.tensor_scalar_add(
                            out=sp[:], in0=ps2[:], scalar1=EPS
                        )
                        rcp = w2.tile([P, H], F32, tag="rcp")
                        nc.vector.reciprocal(out=rcp[:], in_=sp[:])
                        rv = bass.AP(
                            rcp[:].tensor, rcp[:].offset,
                            [rcp[:].ap[0], [1, H], [0, HE]],
                        )
                        htile = w2.tile([P, FD], F32, tag="htile")
                        if l < 2:
                            hr = w2.tile([P, FD], F32, tag="hr")
                            nc.scalar.activation(
                                out=hr[:], in_=ps[:], func=AF.Relu
                            )
                            nc.vector.tensor_tensor(
                                out=htile[:].rearrange("p (h d) -> p h d", h=H),
                                in0=hr[:].rearrange("p (h d) -> p h d", h=H),
                                in1=rv,
                                op=ALU.mult,
                            )
                            pT2 = q2t.tile([FD, P], F32, tag="pT2")
                            nc.tensor.transpose(
                                out=pT2[:], in_=htile[:], identity=ident[:]
                            )
                            nc.vector.tensor_copy(
                                out=nh_slice[0:FD, b * P : (b + 1) * P],
                                in_=pT2[:],
                            )
                        else:
                            nc.vector.tensor_tensor(
                                out=htile[:].rearrange("p (h d) -> p h d", h=H),
                                in0=ps[:].rearrange("p (h d) -> p h d", h=H),
                                in1=rv,
                                op=ALU.mult,
                            )
                            nc.sync.dma_start(
                                out=out3[b * P : (b + 1) * P, :], in_=htile[:]
                            )

                if debug and l == 0:
                    nc.sync.dma_start(out=dbg["d_nh"][:], in_=nh_slice[:])
                if l < 2:
                    tables(nh_slice, wcat_sb[l + 1])

    nc.compile()
    return nc


_CACHE = {}




def run(inputs, cfg, core_ids=None, trace=False, debug=False):
    from concourse.bass_utils import run_bass_kernel_spmd

    percore = host_prep(inputs, cfg)
    key = (cfg.ndev, cfg.nblk, cfg.T, cfg.odf, cfg.g1, debug)
    if key not in _CACHE:
        _CACHE[key] = build_program(cfg, debug=debug)
    nc = _CACHE[key]
    if core_ids is None:
        core_ids = list(range(cfg.ndev))
    res = run_bass_kernel_spmd(nc, percore, core_ids, trace=trace)
    outs = [res.results[i]["out3"] for i in range(cfg.ndev)]
    full = np.concatenate(outs, axis=0)  # [npad, 96]
    return full, res


def make_cfg(inputs):
    dst = np.asarray(inputs["dst"]).astype(np.int64)
    n_real = 50000
    ndev = 8
    nblk = 49
    npad = ndev * nblk * P
    blkcnt = np.bincount(dst // P, minlength=npad // P)
    T = int(np.ceil(blkcnt.max() / P))
    odf = np.asarray(inputs["countyodfeats"]).shape[1]
    return Cfg(ndev, nblk, T, odf, n_real)


def kernel(**inputs) -> np.ndarray:
    cfg = make_cfg(inputs)
    full, _ = run(inputs, cfg)
    idxs = np.asarray(inputs["idxs"]).astype(np.int64)
    return np.ascontiguousarray(full[idxs]).astype(np.float32)



# revision 24
# speedup vs baseline: 1.3563x; 1.3563x over previous
"""Trainium2 Bass kernel for 3-layer EGAT message passing (nn_COUNTYOD).

Strategy (8 cores, edge parallelism by dst range):
  - Host: sort edges by dst; device d owns dst nodes [d*6272, (d+1)*6272);
    edges grouped into 49 blocks of 128 dst nodes. Within each block,
    edges are split by src < 32768 (dma_gather indices are int16): lo
    edges fill tiles [0, T1), hi edges tiles [T1, T1+T2), padded with
    src=0 / dstloc=-1 slots. T1/T2 are global (data max) so the program
    is SPMD-uniform.
  - Tables: catG[n] = [nh@Wni + b | nh@Wns | pad] (N x 256 bf16, 512B
    rows) AllGather'd once per layer; njloc[n] = [nh@Wnj | pad] (R x 128
    bf16) stays device-local (dst indices are local).
  - Per block: three dma_gather calls (src-lo, src-hi from catG halves,
    dst from njloc) batch all row fetches into ~3 Pool instructions.
  - f_pre accumulated on PE in PSUM: per-tile ef@Wfij + identity-lhsT
    injections of gathered ni / nj rows. leaky = 0.01x + 0.99*relu(x)
    (one Act + one DVE op, single PSUM read each); logits via mul +
    grouped reduce on DVE.
  - Scatter transposed: hT[99,128] += matmul(lhsT=[msg|ex], rhs=onehot);
    h and the softmax denominator come from one PSUM tile. Output is
    stored transposed and fixed up on host.
  - Edge features for the next layer stored untransposed (128-col padded
    tiles) and read back with dma_start_transpose (XBAR), avoiding PE
    transposes / PSUM copies on the store path.
"""

import sys
import numpy as np
import ml_dtypes

for _p in ("/opt/trn_rl_repo",):
    if _p not in sys.path:
        sys.path.insert(0, _p)

import concourse.bass as bass  # noqa: E402
import concourse.bacc as bacc  # noqa: E402
import concourse.mybir as mybir  # noqa: E402
import concourse.tile as tile  # noqa: E402

F32 = mybir.dt.float32
BF16 = mybir.dt.bfloat16
I32 = mybir.dt.int32
I16 = mybir.dt.int16
AF = mybir.ActivationFunctionType
ALU = mybir.AluOpType
BF = ml_dtypes.bfloat16

P = 128
FD = 96           # H*HE
CA = 256          # cat row cols ([ni|ns|pad64]), 512B rows
NJ = 128          # njloc row cols ([nj|pad32]), 256B rows
H = 3
HE = 32
EPS = 1e-20
G = 5             # tiles per chunk (psum bank limit: 5*96*4B < 2KB)
LOHI = 32768      # int16 index split


class Cfg:
    def __init__(self, ndev, nblk, T1, T2, odf, n_real):
        self.ndev = ndev
        self.nblk = nblk              # blocks (of 128 dst nodes) per device
        self.T1 = T1                  # lo-src tiles per block
        self.T2 = T2                  # hi-src tiles per block
        self.T = T1 + T2
        self.odf = odf                # layer-1 edge feature dim
        self.n_real = n_real
        self.R = nblk * P             # nodes per device
        self.npad = ndev * self.R
        self.ttot = nblk * self.T     # tiles per device
        self.epd = self.ttot * P      # padded edges per device

    @property
    def key(self):
        return (self.ndev, self.nblk, self.T1, self.T2, self.odf)


def _wrap16(flat, ncores=8):
    """int16 flat index list -> [128, ceil(n/16)] wrapped + core-replicated."""
    n = len(flat)
    n16 = (n + 15) // 16
    a = np.zeros(n16 * 16, np.int16)
    a[:n] = flat
    a = a.reshape(n16, 16).T          # [16, n16]
    return np.tile(a, (ncores, 1))    # [128, n16]


def host_prep(inputs, cfg):
    """Sort/pad/shard on host. Returns per-core input maps."""
    src = np.asarray(inputs["src"]).astype(np.int64)
    dst = np.asarray(inputs["dst"]).astype(np.int64)

    order = np.argsort(dst, kind="stable")
    ssrc, sdst = src[order], dst[order]
    ef0 = np.asarray(inputs["countyodfeats"]).astype(np.float32)[order]

    nb_all = cfg.npad // P
    blkcnt = np.bincount(sdst // P, minlength=nb_all)
    starts = np.zeros(nb_all + 1, np.int64)
    np.cumsum(blkcnt, out=starts[1:])

    T, T1, T2 = cfg.T, cfg.T1, cfg.T2
    TP = T * P
    percore = []
    for d in range(cfg.ndev):
        dev_base = d * cfg.R
        sl_lo = np.zeros((cfg.nblk, T1 * P), np.int16)
        sl_hi = np.zeros((cfg.nblk, T2 * P), np.int16)
        sl_dst = np.zeros((cfg.nblk, TP), np.int16)
        dloc = np.full((cfg.nblk, TP), -1.0, np.float32)
        efp = np.zeros((cfg.nblk, TP, cfg.odf), np.float32)
        for b in range(cfg.nblk):
            g = d * cfg.nblk + b
            s0, s1 = starts[g], starts[g + 1]
            es, ed, ee = ssrc[s0:s1], sdst[s0:s1], ef0[s0:s1]
            lo = es < LOHI
            nlo = int(lo.sum())
            nhi = int((~lo).sum())
            assert nlo <= T1 * P and nhi <= T2 * P
            # slot positions: lo edges 0..nlo-1; hi edges T1*P..T1*P+nhi-1
            pos = np.zeros(len(es), np.int64)
            pos[lo] = np.arange(nlo)
            pos[~lo] = T1 * P + np.arange(nhi)
            sl_lo[b, :nlo] = es[lo].astype(np.int16)
            sl_hi[b, :nhi] = (es[~lo] - LOHI).astype(np.int16)
            sl_dst[b, pos] = (ed - dev_base).astype(np.int16)
            dloc[b, pos] = (ed - g * P).astype(np.float32)
            efp[b, pos] = ee
        idx_lo = np.concatenate([_wrap16(sl_lo[b]) for b in range(cfg.nblk)],
                                axis=1)
        idx_hi = np.concatenate([_wrap16(sl_hi[b]) for b in range(cfg.nblk)],
                                axis=1)
        idx_dst = np.concatenate([_wrap16(sl_dst[b]) for b in range(cfg.nblk)],
                                 axis=1)
        dloc = dloc.reshape(-1, P).T  # [128, ttot]
        percore.append(
            dict(
                idx_lo=np.ascontiguousarray(idx_lo),
                idx_hi=np.ascontiguousarray(idx_hi),
                idx_dst=np.ascontiguousarray(idx_dst),
                dstloc=np.ascontiguousarray(dloc.astype(BF)),
                ef0T=np.ascontiguousarray(
                    efp.reshape(-1, cfg.odf).T.astype(BF)
                ),
            )
        )

    nh0 = np.asarray(inputs["ndata_h"]).astype(np.float32)
    nh0p = np.zeros((cfg.npad, FD), np.float32)
    nh0p[: cfg.n_real] = nh0[: cfg.n_real]
    for d in range(cfg.ndev):
        sl = nh0p[d * cfg.R : (d + 1) * cfg.R]
        percore[d]["nh0T"] = np.ascontiguousarray(
            np.concatenate([sl.T, np.ones((1, cfg.R), np.float32)], axis=0)
            .astype(BF)
        )

    def wcat_ext(Wni, Wns, bias):
        w = np.zeros((FD + 1, CA), np.float32)
        w[:FD, 0:FD] = Wni
        w[:FD, FD:2 * FD] = Wns
        w[FD, 0:FD] = bias
        return np.ascontiguousarray(w.astype(BF))

    def wnj_ext(Wnj):
        w = np.zeros((FD + 1, NJ), np.float32)
        w[:FD, 0:FD] = Wnj
        return np.ascontiguousarray(w.astype(BF))

    g = lambda k: np.asarray(inputs[k]).astype(np.float32)
    e3 = np.zeros((H, FD), np.float32)
    for h in range(H):
        e3[h, h * HE : (h + 1) * HE] = 1.0
    weights = dict(
        wcat0=wcat_ext(g("Wni0"), g("Wns0"), g("bias0")),
        wcat1=wcat_ext(g("Wni")[0], g("Wns")[0], g("bias")[0]),
        wcat2=wcat_ext(g("Wni")[1], g("Wns")[1], g("bias")[1]),
        wnj0=wnj_ext(g("Wnj0")),
        wnj1=wnj_ext(g("Wnj")[0]),
        wnj2=wnj_ext(g("Wnj")[1]),
        wfij0=np.ascontiguousarray(g("Wfij0").astype(BF)),
        wfij1=np.ascontiguousarray(g("Wfij")[0].astype(BF)),
        wfij2=np.ascontiguousarray(g("Wfij")[1].astype(BF)),
        attn0=np.ascontiguousarray(
            np.repeat(g("attn0").reshape(1, FD), P, 0).astype(BF)),
        attn1=np.ascontiguousarray(
            np.repeat(g("attn").reshape(2, FD)[0:1], P, 0).astype(BF)),
        attn2=np.ascontiguousarray(
            np.repeat(g("attn").reshape(2, FD)[1:2], P, 0).astype(BF)),
        e3=np.ascontiguousarray(e3),
    )
    for d in range(cfg.ndev):
        percore[d].update(weights)
    return percore


def build_program(cfg, debug=False, reps=1):
    nc = bacc.Bacc("TRN2", target_bir_lowering=False, debug=False)
    c = cfg
    Ttot, EPD = c.ttot, c.epd
    T, T1, T2 = c.T, c.T1, c.T2
    NLO = min(LOHI, c.npad)
    NHI = max(c.npad - LOHI, 1)

    # ---- I/O ----
    pr = {}
    pr["nh0T"] = nc.declare_dram_parameter("nh0T", [FD + 1, c.R], BF16, isOutput=False)
    pr["ef0T"] = nc.declare_dram_parameter("ef0T", [c.odf, EPD], BF16, isOutput=False)
    pr["idx_lo"] = nc.declare_dram_parameter("idx_lo", [P, c.nblk * T1 * 8], I16, isOutput=False)
    pr["idx_hi"] = nc.declare_dram_parameter("idx_hi", [P, c.nblk * T2 * 8], I16, isOutput=False)
    pr["idx_dst"] = nc.declare_dram_parameter("idx_dst", [P, c.nblk * T * 8], I16, isOutput=False)
    pr["dstloc"] = nc.declare_dram_parameter("dstloc", [P, Ttot], BF16, isOutput=False)
    for i in range(3):
        pr[f"wcat{i}"] = nc.declare_dram_parameter(f"wcat{i}", [FD + 1, CA], BF16, isOutput=False)
        pr[f"wnj{i}"] = nc.declare_dram_parameter(f"wnj{i}", [FD + 1, NJ], BF16, isOutput=False)
        pr[f"attn{i}"] = nc.declare_dram_parameter(f"attn{i}", [P, FD], BF16, isOutput=False)
    pr["wfij0"] = nc.declare_dram_parameter("wfij0", [c.odf, FD], BF16, isOutput=False)
    pr["wfij1"] = nc.declare_dram_parameter("wfij1", [FD, FD], BF16, isOutput=False)
    pr["wfij2"] = nc.declare_dram_parameter("wfij2", [FD, FD], BF16, isOutput=False)
    pr["e3"] = nc.declare_dram_parameter("e3", [H, FD], F32, isOutput=False)
    out3T = nc.declare_dram_parameter("out3T", [FD, c.R], F32, isOutput=True)
    dbg = {}
    if debug:
        dbg["d_cat"] = nc.declare_dram_parameter("d_cat", [c.npad, CA], BF16, isOutput=True)
        dbg["d_e"] = nc.declare_dram_parameter("d_e", [P, T * H], F32, isOutput=True)
        dbg["d_msg"] = nc.declare_dram_parameter("d_msg", [P, T * (FD + H)], BF16, isOutput=True)
        dbg["d_oh"] = nc.declare_dram_parameter("d_oh", [P, T * P], BF16, isOutput=True)
        dbg["d_gatA"] = nc.declare_dram_parameter("d_gatA", [P, T * CA], BF16, isOutput=True)
        dbg["d_gatB"] = nc.declare_dram_parameter("d_gatB", [P, T * NJ], BF16, isOutput=True)
        dbg["d_nh"] = nc.declare_dram_parameter("d_nh", [FD + 1, c.R], BF16, isOutput=True)
        dbg["d_fp"] = nc.declare_dram_parameter("d_fp", [P, G * FD], F32, isOutput=True)
        dbg["d_hT"] = nc.declare_dram_parameter("d_hT", [FD + H, P], F32, isOutput=True)

    # ---- internal DRAM ----
    f1 = nc.dram_tensor("f1", [P, Ttot * P], BF16)
    f2 = nc.dram_tensor("f2", [P, Ttot * P], BF16)
    catL = nc.dram_tensor("catL", [c.R, CA], BF16)
    njloc = nc.dram_tensor("njloc", [c.R, NJ], BF16)
    catGa = nc.dram_tensor("catGa", [c.npad, CA], BF16, addr_space="Shared")
    catGb = nc.dram_tensor("catGb", [c.npad, CA], BF16, addr_space="Shared")

    rg = [list(range(c.ndev))]

    chunks = []
    cc0 = 0
    while cc0 < T:
        chunks.append((cc0, min(G, T - cc0)))
        cc0 += G

    with tile.TileContext(nc) as tc:
        with tc.tile_pool(name="persist", bufs=1) as pp:
            iota_i = pp.tile([P, P], I32, tag="iota_i")
            nc.gpsimd.iota(iota_i[:], pattern=[[1, P]], base=0, channel_multiplier=0)
            iota_b = pp.tile([P, P], BF16, tag="iota_b")
            nc.vector.tensor_copy(out=iota_b[:], in_=iota_i[:])
            ident = pp.tile([P, P], BF16, tag="ident")
            iota_c = pp.tile([P, 1], I32, tag="iota_c")
            nc.gpsimd.iota(iota_c[:], pattern=[[1, 1]], base=0, channel_multiplier=1)
            iota_cb = pp.tile([P, 1], BF16, tag="iota_cb")
            nc.vector.tensor_copy(out=iota_cb[:], in_=iota_c[:])
            nc.vector.tensor_tensor(
                out=ident[:],
                in0=iota_b[:],
                in1=bass.AP(iota_cb[:].tensor, iota_cb[:].offset,
                            [iota_cb[:].ap[0], [0, P]]),
                op=ALU.is_equal,
            )

            ilo = pp.tile([P, c.nblk * T1 * 8], I16, tag="ilo")
            ihi = pp.tile([P, c.nblk * T2 * 8], I16, tag="ihi")
            idst = pp.tile([P, c.nblk * T * 8], I16, tag="idst")
            dstloc = pp.tile([P, Ttot], BF16, tag="dstloc")
            nc.sync.dma_start(out=ilo[:], in_=pr["idx_lo"][:])
            nc.sync.dma_start(out=ihi[:], in_=pr["idx_hi"][:])
            nc.sync.dma_start(out=idst[:], in_=pr["idx_dst"][:])
            nc.sync.dma_start(out=dstloc[:], in_=pr["dstloc"][:])

            wfij_sb, wcat_sb, wnj_sb, attn_sb = [], [], [], []
            for l in range(3):
                cdim = c.odf if l == 0 else FD
                wf = pp.tile([cdim, FD], BF16, tag=f"wfij{l}")
                nc.sync.dma_start(out=wf[:], in_=pr[f"wfij{l}"][:])
                wfij_sb.append(wf)
                wc = pp.tile([FD + 1, CA], BF16, tag=f"wcat{l}")
                nc.sync.dma_start(out=wc[:], in_=pr[f"wcat{l}"][:])
                wcat_sb.append(wc)
                wn = pp.tile([FD + 1, NJ], BF16, tag=f"wnj{l}")
                nc.sync.dma_start(out=wn[:], in_=pr[f"wnj{l}"][:])
                wnj_sb.append(wn)
                ab = pp.tile([P, FD], BF16, tag=f"attn{l}")
                nc.sync.dma_start(out=ab[:], in_=pr[f"attn{l}"][:])
                attn_sb.append(ab)
            e3_sb = pp.tile([H, FD], F32, tag="e3")
            nc.sync.dma_start(out=e3_sb[:], in_=pr["e3"][:])

            nh_slice = pp.tile([FD + 1, c.R], BF16, tag="nh_slice")

            def tables_block(qC, wC, b, wc, wn):
                ct = qC.tile([P, CA], F32, tag="ct")
                nc.tensor.matmul(
                    out=ct[:],
                    lhsT=nh_slice[:, b * P : (b + 1) * P],
                    rhs=wc[:],
                    start=True,
                    stop=True,
                )
                cs = wC.tile([P, CA], BF16, tag="cs")
                nc.scalar.activation(out=cs[:], in_=ct[:], func=AF.Copy)
                nc.sync.dma_start(out=catL[b * P : (b + 1) * P, :], in_=cs[:])
                ctn = qC.tile([P, NJ], F32, tag="ctn")
                nc.tensor.matmul(
                    out=ctn[:],
                    lhsT=nh_slice[:, b * P : (b + 1) * P],
                    rhs=wn[:],
                    start=True,
                    stop=True,
                )
                cn = wC.tile([P, NJ], BF16, tag="cn")
                nc.scalar.activation(out=cn[:], in_=ctn[:], func=AF.Copy)
                nc.sync.dma_start(out=njloc[b * P : (b + 1) * P, :], in_=cn[:])

            for rep in range(reps):
              # ---- layer 0 tables prologue ----
              nc.sync.dma_start(out=nh_slice[:], in_=pr["nh0T"][:])
              with (
                tc.tile_pool(name=f"t0_{rep}", bufs=2) as wC0,
                tc.tile_pool(name=f"t0p_{rep}", bufs=2, space="PSUM") as qC0,
              ):
                for b in range(c.nblk):
                    tables_block(qC0, wC0, b, wcat_sb[0], wnj_sb[0])
              nc.gpsimd.collective_compute(
                "AllGather", ALU.bypass, replica_groups=rg,
                ins=[catL[:]], outs=[catGa[:]],
              )

              for l in range(3):
                cdim = c.odf if l == 0 else FD
                catG = catGa if l % 2 == 0 else catGb
                catG_next = catGb if l % 2 == 0 else catGa
                f_src = None if l == 0 else (f1 if l == 1 else f2)
                f_dst = f1 if l == 0 else (f2 if l == 1 else None)
                store_f = l < 2
                wf = wfij_sb[l]
                ab = attn_sb[l]
                catLoV = bass.AP(catG[:].tensor, 0, [[CA, NLO], [1, CA]])
                if c.npad > LOHI:
                    catHiV = bass.AP(catG[:].tensor, LOHI * CA,
                                     [[CA, NHI], [1, CA]])
                else:
                    catHiV = catLoV

                if debug and l == 0:
                    nc.sync.dma_start(out=dbg["d_cat"][:], in_=catGa[:])
                if debug and l == 1:
                    nc.sync.dma_start(out=dbg["d_nh"][:], in_=nh_slice[:])
                with (
                    tc.tile_pool(name=f"pA{l}_{rep}", bufs=2) as pA,
                    tc.tile_pool(name=f"pB{l}_{rep}", bufs=3) as pB,
                    tc.tile_pool(name=f"qF{l}_{rep}", bufs=2, space="PSUM") as qF,
                    tc.tile_pool(name=f"qH{l}_{rep}", bufs=2, space="PSUM") as qH,
                    tc.tile_pool(name=f"qC{l}_{rep}", bufs=1, space="PSUM") as qC,
                ):
                    def gather_rows(gtile, col0, in_ap, itile, icol0, ntiles,
                                    elem):
                        # SWDGE ring limit: <=1024 indices (8 tiles) per inst
                        done = 0
                        while done < ntiles:
                            k = min(8, ntiles - done)
                            nc.gpsimd.dma_gather(
                                out_ap=bass.AP(
                                    gtile[:].tensor,
                                    gtile[:].offset + (col0 + done) * elem,
                                    [gtile[:].ap[0], [elem, k], [1, elem]]),
                                in_ap=in_ap,
                                idxs_ap=itile[:, icol0 + done * 8
                                              : icol0 + (done + k) * 8],
                                num_idxs=k * P,
                                num_idxs_reg=k * P,
                                elem_size=elem,
                            )
                            done += k

                    for b in range(c.nblk):
                        t0 = b * T
                        # ---- block-level gathers ----
                        gatA = pA.tile([P, T * CA], BF16, tag="gatA")
                        gather_rows(gatA, 0, catLoV, ilo, b * T1 * 8, T1, CA)
                        gather_rows(gatA, T1, catHiV, ihi, b * T2 * 8, T2, CA)
                        gatB = pA.tile([P, T * NJ], BF16, tag="gatB")
                        gather_rows(gatB, 0, njloc[:, :], idst, b * T * 8, T, NJ)
                        if l == 0:
                            efc = pA.tile([c.odf, T * P], BF16, tag="efc")
                            nc.sync.dma_start(
                                out=efc[:],
                                in_=pr["ef0T"][:, t0 * P : (t0 + T) * P],
                            )
                        else:
                            efc = pA.tile([P, T * P], BF16, tag="efc")
                            nc.sync.dma_start_transpose(
                                out=efc[:].rearrange("p (t q) -> p t q", t=T),
                                in_=bass.AP(
                                    f_src[:].tensor, t0 * P,
                                    [f_src[:].ap[0], [1, T * P]],
                                ),
                            )
                        # one-hot [p_edge, (t, v)]
                        ohE = pA.tile([P, T * P], BF16, tag="ohE")
                        iov = bass.AP(iota_b[:].tensor, iota_b[:].offset,
                                      [iota_b[:].ap[0], [0, T], [1, P]])
                        dlv = bass.AP(dstloc[:].tensor, dstloc[:].offset + t0,
                                      [dstloc[:].ap[0], [1, T], [0, P]])
                        nc.vector.tensor_tensor(
                            out=ohE[:].rearrange("p (t v) -> p t v", t=T),
                            in0=iov, in1=dlv, op=ALU.is_equal,
                        )
                        e_sb = pA.tile([P, T * H], F32, tag="e_sb")
                        msg = pA.tile([P, T * (FD + H)], BF16, tag="msg")
                        if store_f:
                            frB = pA.tile([P, T * FD], BF16, tag="frB")
                        hT = qH.tile([FD + H, P], F32, tag="hT")

                        # ---- pass A: f_pre, leaky, logits ----
                        for (cc, g) in chunks:
                            W = g * FD
                            fp = qF.tile([P, G * FD], F32, tag="fp")
                            ni_v = bass.AP(
                                gatA[:].tensor, gatA[:].offset + cc * CA,
                                [gatA[:].ap[0], [CA, g], [1, FD]],
                            )
                            nc.tensor.matmul(
                                out=fp[:, 0:W], lhsT=ident[:], rhs=ni_v,
                                start=True, stop=False, skip_group_check=True,
                            )
                            nj_v = bass.AP(
                                gatB[:].tensor, gatB[:].offset + cc * NJ,
                                [gatB[:].ap[0], [NJ, g], [1, FD]],
                            )
                            nc.tensor.matmul(
                                out=fp[:, 0:W], lhsT=ident[:], rhs=nj_v,
                                start=False, stop=False, skip_group_check=True,
                            )
                            for j in range(g):
                                nc.tensor.matmul(
                                    out=fp[:, j * FD : (j + 1) * FD],
                                    lhsT=efc[0:cdim, (cc + j) * P : (cc + j + 1) * P],
                                    rhs=wf[:],
                                    start=False, stop=True,
                                    skip_group_check=True,
                                )
                            if store_f:
                                nc.scalar.activation(
                                    out=frB[:, cc * FD : (cc + g) * FD],
                                    in_=fp[:, 0:W], func=AF.Relu,
                                )
                            # leaky(x) = 0.01*x + 0.99*relu(x)
                            a1 = pB.tile([P, G * FD], BF16, tag="a1")
                            nc.scalar.activation(
                                out=a1[:, 0:W], in_=fp[:, 0:W],
                                func=AF.Relu, scale=0.99,
                            )
                            fl = pB.tile([P, G * FD], BF16, tag="fl")
                            nc.vector.scalar_tensor_tensor(
                                out=fl[:, 0:W], in0=fp[:, 0:W], scalar=0.01,
                                in1=a1[:, 0:W], op0=ALU.mult, op1=ALU.add,
                            )
                            if debug and l == 0 and b == 0 and cc == 0:
                                dfp = pB.tile([P, G * FD], F32, tag="dfp")
                                nc.scalar.activation(
                                    out=dfp[:], in_=fp[:], func=AF.Copy)
                                nc.sync.dma_start(out=dbg["d_fp"][:], in_=dfp[:])
                            scrm = pB.tile([P, G * FD], BF16, tag="scrm")
                            fl_v = bass.AP(
                                fl[:].tensor, fl[:].offset,
                                [fl[:].ap[0], [FD, g], [HE, H], [1, HE]],
                            )
                            a_v = bass.AP(
                                ab[:].tensor, ab[:].offset,
                                [ab[:].ap[0], [0, g], [HE, H], [1, HE]],
                            )
                            nc.vector.tensor_tensor(
                                out=bass.AP(
                                    scrm[:].tensor, scrm[:].offset,
                                    [scrm[:].ap[0], [FD, g], [HE, H], [1, HE]],
                                ),
                                in0=fl_v, in1=a_v, op=ALU.mult,
                            )
                            nc.vector.tensor_reduce(
                                out=bass.AP(
                                    e_sb[:].tensor, e_sb[:].offset + cc * H,
                                    [e_sb[:].ap[0], [H, g], [1, H]],
                                ),
                                in_=bass.AP(
                                    scrm[:].tensor, scrm[:].offset,
                                    [scrm[:].ap[0], [FD, g], [HE, H], [1, HE]],
                                ),
                                axis=mybir.AxisListType.X, op=ALU.add,
                            )
                        if store_f:
                            fd_v = bass.AP(
                                f_dst[:].tensor, t0 * P,
                                [f_dst[:].ap[0], [P, T], [1, FD]],
                            )
                            nc.sync.dma_start(out=fd_v, in_=frB[:])

                        # ---- softmax pieces ----
                        nc.vector.tensor_scalar(
                            out=e_sb[:], in0=e_sb[:], scalar1=60.0,
                            scalar2=None, op0=ALU.min,
                        )
                        nc.scalar.activation(
                            out=bass.AP(
                                msg[:].tensor, msg[:].offset + FD,
                                [msg[:].ap[0], [FD + H, T], [1, H]],
                            ),
                            in_=e_sb[:].rearrange("p (t h) -> p t h", t=T),
                            func=AF.Exp,
                        )
                        ns_v = bass.AP(
                            gatA[:].tensor, gatA[:].offset + FD,
                            [gatA[:].ap[0], [CA, T], [HE, H], [1, HE]],
                        )
                        ex_v = bass.AP(
                            msg[:].tensor, msg[:].offset + FD,
                            [msg[:].ap[0], [FD + H, T], [1, H], [0, HE]],
                        )
                        nc.vector.tensor_tensor(
                            out=bass.AP(
                                msg[:].tensor, msg[:].offset,
                                [msg[:].ap[0], [FD + H, T], [HE, H], [1, HE]],
                            ),
                            in0=ns_v, in1=ex_v, op=ALU.mult,
                        )
                        if debug and l == 0 and b == 0:
                            nc.sync.dma_start(out=dbg["d_e"][:], in_=e_sb[:])
                            nc.sync.dma_start(out=dbg["d_msg"][:], in_=msg[:])
                            nc.sync.dma_start(out=dbg["d_oh"][:], in_=ohE[:])
                            nc.sync.dma_start(out=dbg["d_gatA"][:], in_=gatA[:])
                            nc.sync.dma_start(out=dbg["d_gatB"][:], in_=gatB[:])
                        # ---- scatter ----
                        for t in range(T):
                            nc.tensor.matmul(
                                out=hT[:],
                                lhsT=msg[:, t * (FD + H) : (t + 1) * (FD + H)],
                                rhs=ohE[:, t * P : (t + 1) * P],
                                start=(t == 0), stop=(t == T - 1),
                                skip_group_check=True,
                            )
                        if debug and l == 0 and b == 0:
                            dhT = pB.tile([FD + H, P], F32, tag="dhT")
                            nc.scalar.activation(
                                out=dhT[:], in_=hT[:], func=AF.Copy)
                            nc.sync.dma_start(out=dbg["d_hT"][:], in_=dhT[:])
                        # ---- finalize ----
                        srec = pB.tile([H, P], F32, tag="srec")
                        nc.vector.tensor_scalar_add(
                            out=srec[:], in0=hT[FD : FD + H, :], scalar1=EPS,
                        )
                        rcp = pB.tile([H, P], F32, tag="rcp")
                        nc.vector.reciprocal(out=rcp[:], in_=srec[:])
                        rcp96 = qC.tile([FD, P], F32, tag="rcp96")
                        nc.tensor.matmul(
                            out=rcp96[:], lhsT=e3_sb[:], rhs=rcp[:],
                            start=True, stop=True,
                        )
                        rcp96s = pB.tile([FD, P], F32, tag="rcp96s")
                        nc.scalar.activation(
                            out=rcp96s[:], in_=rcp96[:], func=AF.Copy,
                        )
                        if l < 2:
                            nc.vector.scalar_tensor_tensor(
                                out=nh_slice[0:FD, b * P : (b + 1) * P],
                                in0=hT[0:FD, :], scalar=0.0,
                                in1=rcp96s[:], op0=ALU.max, op1=ALU.mult,
                            )
                            tables_block(qC, pB, b, wcat_sb[l + 1], wnj_sb[l + 1])
                        else:
                            hfin = pB.tile([FD, P], F32, tag="hfin")
                            nc.vector.scalar_tensor_tensor(
                                out=hfin[:], in0=hT[0:FD, :], scalar=1.0,
                                in1=rcp96s[:], op0=ALU.mult, op1=ALU.mult,
                            )
                            nc.sync.dma_start(
                                out=out3T[:, b * P : (b + 1) * P], in_=hfin[:],
                            )

                if l < 2:
                    nc.gpsimd.collective_compute(
                        "AllGather", ALU.bypass, replica_groups=rg,
                        ins=[catL[:]], outs=[catG_next[:]],
                    )

    nc.compile()
    return nc


_CACHE = {}


def get_program(cfg, reps=1):
    k = cfg.key + (reps,)
    if k not in _CACHE:
        _CACHE[k] = build_program(cfg, reps=reps)
    return _CACHE[k]


def run(inputs, cfg, core_ids=None, trace=False):
    from concourse.bass_utils import run_bass_kernel_spmd

    percore = host_prep(inputs, cfg)
    nc = get_program(cfg)
    if core_ids is None:
        core_ids = list(range(cfg.ndev))
    res = run_bass_kernel_spmd(nc, percore, core_ids, trace=trace)
    outs = [res.results[i]["out3T"].T for i in range(cfg.ndev)]
    full = np.concatenate(outs, axis=0)  # [npad, 96]
    return full, res


def make_cfg(inputs):
    src = np.asarray(inputs["src"]).astype(np.int64)
    dst = np.asarray(inputs["dst"]).astype(np.int64)
    n_real = 50000
    ndev = 8
    nblk = 49
    npad = ndev * nblk * P
    nb_all = npad // P
    blk = dst // P
    lo = src < LOHI
    cnt_lo = np.bincount(blk[lo], minlength=nb_all)
    cnt_hi = np.bincount(blk[~lo], minlength=nb_all)
    T1 = max(1, int(np.ceil(cnt_lo.max() / P)))
    T2 = max(1, int(np.ceil(cnt_hi.max() / P)))
    odf = np.asarray(inputs["countyodfeats"]).shape[1]
    return Cfg(ndev, nblk, T1, T2, odf, n_real)


def kernel(**inputs) -> np.ndarray:
    cfg = make_cfg(inputs)
    full, _ = run(inputs, cfg)
    idxs = np.asarray(inputs["idxs"]).astype(np.int64)
    return np.ascontiguousarray(full[idxs]).astype(np.float32)


# revision 27
# speedup vs baseline: 1.6695x; 1.2310x over previous
"""Trainium2 Bass kernel for 3-layer EGAT message passing (nn_COUNTYOD).

Strategy (8 cores, edge parallelism by dst range):
  - Host: sort edges by dst; device d owns dst nodes [d*6272, (d+1)*6272);
    edges grouped into 49 blocks of 128 dst nodes. Within each block,
    edges are split by src < 32768 (dma_gather indices are int16): lo
    edges fill tiles [0, T1), hi edges tiles [T1, T1+T2), padded with
    src=0 / dstloc=-1 slots. T1/T2 are global (data max) so the program
    is SPMD-uniform.
  - Tables: catG[n] = [nh@Wni + b | nh@Wns | pad] (N x 256 bf16, 512B
    rows) AllGather'd once per layer; njloc[n] = [nh@Wnj | pad] (R x 128
    bf16) stays device-local (dst indices are local).
  - Per block: three dma_gather calls (src-lo, src-hi from catG halves,
    dst from njloc) batch all row fetches into ~3 Pool instructions.
  - f_pre accumulated on PE in PSUM: per-tile ef@Wfij + identity-lhsT
    injections of gathered ni / nj rows. leaky = 0.01x + 0.99*relu(x)
    (one Act + one DVE op, single PSUM read each); logits via mul +
    grouped reduce on DVE.
  - Scatter transposed: hT[99,128] += matmul(lhsT=[msg|ex], rhs=onehot);
    h and the softmax denominator come from one PSUM tile. Output is
    stored transposed and fixed up on host.
  - Edge features for the next layer stored untransposed (128-col padded
    tiles) and read back with dma_start_transpose (XBAR), avoiding PE
    transposes / PSUM copies on the store path.
"""

import sys
import numpy as np
import ml_dtypes

for _p in ("/opt/trn_rl_repo",):
    if _p not in sys.path:
        sys.path.insert(0, _p)

import concourse.bass as bass  # noqa: E402
import concourse.bacc as bacc  # noqa: E402
import concourse.mybir as mybir  # noqa: E402
import concourse.tile as tile  # noqa: E402

F32 = mybir.dt.float32
BF16 = mybir.dt.bfloat16
I32 = mybir.dt.int32
I16 = mybir.dt.int16
AF = mybir.ActivationFunctionType
ALU = mybir.AluOpType
BF = ml_dtypes.bfloat16

P = 128
FD = 96           # H*HE
CA = 256          # cat row cols ([ni|ns|pad64]), 512B rows
NJ = 128          # njloc row cols ([nj|pad32]), 256B rows
H = 3
HE = 32
EPS = 1e-20
G = 5             # tiles per chunk (psum bank limit: 5*96*4B < 2KB)
LOHI = 32768      # int16 index split


class Cfg:
    def __init__(self, ndev, nblk, T1, T2, odf, n_real):
        self.ndev = ndev
        self.nblk = nblk              # blocks (of 128 dst nodes) per device
        self.T1 = T1                  # lo-src tiles per block
        self.T2 = T2                  # hi-src tiles per block
        self.T = T1 + T2
        self.odf = odf                # layer-1 edge feature dim
        self.n_real = n_real
        self.R = nblk * P             # nodes per device
        self.npad = ndev * self.R
        self.ttot = nblk * self.T     # tiles per device
        self.epd = self.ttot * P      # padded edges per device

    @property
    def key(self):
        return (self.ndev, self.nblk, self.T1, self.T2, self.odf)


def _wrap16(flat, ncores=8):
    """int16 flat index list -> [128, ceil(n/16)] wrapped + core-replicated."""
    n = len(flat)
    n16 = (n + 15) // 16
    a = np.zeros(n16 * 16, np.int16)
    a[:n] = flat
    a = a.reshape(n16, 16).T          # [16, n16]
    return np.tile(a, (ncores, 1))    # [128, n16]


def host_prep(inputs, cfg):
    """Sort/pad/shard on host. Returns per-core input maps."""
    src = np.asarray(inputs["src"]).astype(np.int64)
    dst = np.asarray(inputs["dst"]).astype(np.int64)

    order = np.argsort(dst, kind="stable")
    ssrc, sdst = src[order], dst[order]
    ef0 = np.asarray(inputs["countyodfeats"]).astype(np.float32)[order]

    nb_all = cfg.npad // P
    blkcnt = np.bincount(sdst // P, minlength=nb_all)
    starts = np.zeros(nb_all + 1, np.int64)
    np.cumsum(blkcnt, out=starts[1:])

    T, T1, T2 = cfg.T, cfg.T1, cfg.T2
    TP = T * P
    percore = []
    for d in range(cfg.ndev):
        dev_base = d * cfg.R
        sl_lo = np.zeros((cfg.nblk, T1 * P), np.int16)
        sl_hi = np.zeros((cfg.nblk, T2 * P), np.int16)
        sl_dst = np.zeros((cfg.nblk, TP), np.int16)
        dloc = np.full((cfg.nblk, TP), -1.0, np.float32)
        efp = np.zeros((cfg.nblk, TP, cfg.odf), np.float32)
        for b in range(cfg.nblk):
            g = d * cfg.nblk + b
            s0, s1 = starts[g], starts[g + 1]
            es, ed, ee = ssrc[s0:s1], sdst[s0:s1], ef0[s0:s1]
            lo = es < LOHI
            nlo = int(lo.sum())
            nhi = int((~lo).sum())
            assert nlo <= T1 * P and nhi <= T2 * P
            # slot positions: lo edges 0..nlo-1; hi edges T1*P..T1*P+nhi-1
            pos = np.zeros(len(es), np.int64)
            pos[lo] = np.arange(nlo)
            pos[~lo] = T1 * P + np.arange(nhi)
            sl_lo[b, :nlo] = es[lo].astype(np.int16)
            sl_hi[b, :nhi] = (es[~lo] - LOHI).astype(np.int16)
            sl_dst[b, pos] = (ed - dev_base).astype(np.int16)
            dloc[b, pos] = (ed - g * P).astype(np.float32)
            efp[b, pos] = ee
        idx_lo = np.concatenate([_wrap16(sl_lo[b]) for b in range(cfg.nblk)],
                                axis=1)
        idx_hi = np.concatenate([_wrap16(sl_hi[b]) for b in range(cfg.nblk)],
                                axis=1)
        idx_dst = np.concatenate([_wrap16(sl_dst[b]) for b in range(cfg.nblk)],
                                 axis=1)
        dloc = dloc.reshape(-1, P).T  # [128, ttot]
        percore.append(
            dict(
                idx_lo=np.ascontiguousarray(idx_lo),
                idx_hi=np.ascontiguousarray(idx_hi),
                idx_dst=np.ascontiguousarray(idx_dst),
                dstloc=np.ascontiguousarray(dloc.astype(BF)),
                ef0T=np.ascontiguousarray(
                    efp.reshape(-1, cfg.odf).T.astype(BF)
                ),
            )
        )

    nh0 = np.asarray(inputs["ndata_h"]).astype(np.float32)
    nh0p = np.zeros((cfg.npad, FD), np.float32)
    nh0p[: cfg.n_real] = nh0[: cfg.n_real]
    for d in range(cfg.ndev):
        sl = nh0p[d * cfg.R : (d + 1) * cfg.R]
        percore[d]["nh0T"] = np.ascontiguousarray(
            np.concatenate([sl.T, np.ones((1, cfg.R), np.float32)], axis=0)
            .astype(BF)
        )

    def wcat_ext(Wni, Wns, bias):
        w = np.zeros((FD + 1, CA), np.float32)
        w[:FD, 0:FD] = Wni
        w[:FD, FD:2 * FD] = Wns
        w[FD, 0:FD] = bias
        return np.ascontiguousarray(w.astype(BF))

    def wnj_ext(Wnj):
        w = np.zeros((FD + 1, NJ), np.float32)
        w[:FD, 0:FD] = Wnj
        return np.ascontiguousarray(w.astype(BF))

    g = lambda k: np.asarray(inputs[k]).astype(np.float32)
    e3 = np.zeros((H, FD), np.float32)
    for h in range(H):
        e3[h, h * HE : (h + 1) * HE] = 1.0
    weights = dict(
        wcat0=wcat_ext(g("Wni0"), g("Wns0"), g("bias0")),
        wcat1=wcat_ext(g("Wni")[0], g("Wns")[0], g("bias")[0]),
        wcat2=wcat_ext(g("Wni")[1], g("Wns")[1], g("bias")[1]),
        wnj0=wnj_ext(g("Wnj0")),
        wnj1=wnj_ext(g("Wnj")[0]),
        wnj2=wnj_ext(g("Wnj")[1]),
        wfij0=np.ascontiguousarray(g("Wfij0").astype(BF)),
        wfij1=np.ascontiguousarray(g("Wfij")[0].astype(BF)),
        wfij2=np.ascontiguousarray(g("Wfij")[1].astype(BF)),
        attn0=np.ascontiguousarray(
            np.repeat(g("attn0").reshape(1, FD), P, 0).astype(BF)),
        attn1=np.ascontiguousarray(
            np.repeat(g("attn").reshape(2, FD)[0:1], P, 0).astype(BF)),
        attn2=np.ascontiguousarray(
            np.repeat(g("attn").reshape(2, FD)[1:2], P, 0).astype(BF)),
        e3=np.ascontiguousarray(e3),
    )
    for d in range(cfg.ndev):
        percore[d].update(weights)
    return percore


def build_program(cfg, debug=False, reps=1, ablate=frozenset(), nq=2,
                  scratch=32768):
    nc = bacc.Bacc("TRN2", target_bir_lowering=False, debug=False,
                   dynamic_dma_scratch_size=scratch, num_swdge_queues=nq)
    c = cfg
    Ttot, EPD = c.ttot, c.epd
    T, T1, T2 = c.T, c.T1, c.T2
    NLO = min(LOHI, c.npad)
    NHI = max(c.npad - LOHI, 1)

    # ---- I/O ----
    pr = {}
    pr["nh0T"] = nc.declare_dram_parameter("nh0T", [FD + 1, c.R], BF16, isOutput=False)
    pr["ef0T"] = nc.declare_dram_parameter("ef0T", [c.odf, EPD], BF16, isOutput=False)
    pr["idx_lo"] = nc.declare_dram_parameter("idx_lo", [P, c.nblk * T1 * 8], I16, isOutput=False)
    pr["idx_hi"] = nc.declare_dram_parameter("idx_hi", [P, c.nblk * T2 * 8], I16, isOutput=False)
    pr["idx_dst"] = nc.declare_dram_parameter("idx_dst", [P, c.nblk * T * 8], I16, isOutput=False)
    pr["dstloc"] = nc.declare_dram_parameter("dstloc", [P, Ttot], BF16, isOutput=False)
    for i in range(3):
        pr[f"wcat{i}"] = nc.declare_dram_parameter(f"wcat{i}", [FD + 1, CA], BF16, isOutput=False)
        pr[f"wnj{i}"] = nc.declare_dram_parameter(f"wnj{i}", [FD + 1, NJ], BF16, isOutput=False)
        pr[f"attn{i}"] = nc.declare_dram_parameter(f"attn{i}", [P, FD], BF16, isOutput=False)
    pr["wfij0"] = nc.declare_dram_parameter("wfij0", [c.odf, FD], BF16, isOutput=False)
    pr["wfij1"] = nc.declare_dram_parameter("wfij1", [FD, FD], BF16, isOutput=False)
    pr["wfij2"] = nc.declare_dram_parameter("wfij2", [FD, FD], BF16, isOutput=False)
    pr["e3"] = nc.declare_dram_parameter("e3", [H, FD], F32, isOutput=False)
    out3T = nc.declare_dram_parameter("out3T", [FD, c.R], F32, isOutput=True)
    dbg = {}
    if debug:
        dbg["d_cat"] = nc.declare_dram_parameter("d_cat", [c.npad, CA], BF16, isOutput=True)
        dbg["d_e"] = nc.declare_dram_parameter("d_e", [P, T * H], F32, isOutput=True)
        dbg["d_msg"] = nc.declare_dram_parameter("d_msg", [P, T * (FD + H)], BF16, isOutput=True)
        dbg["d_oh"] = nc.declare_dram_parameter("d_oh", [P, T * P], BF16, isOutput=True)
        dbg["d_gatA"] = nc.declare_dram_parameter("d_gatA", [P, T * CA], BF16, isOutput=True)
        dbg["d_gatB"] = nc.declare_dram_parameter("d_gatB", [P, T * NJ], BF16, isOutput=True)
        dbg["d_nh"] = nc.declare_dram_parameter("d_nh", [FD + 1, c.R], BF16, isOutput=True)
        dbg["d_fp"] = nc.declare_dram_parameter("d_fp", [P, G * FD], F32, isOutput=True)
        dbg["d_hT"] = nc.declare_dram_parameter("d_hT", [FD + H, P], F32, isOutput=True)

    # ---- internal DRAM ----
    f1 = nc.dram_tensor("f1", [P, Ttot * P], BF16)
    f2 = nc.dram_tensor("f2", [P, Ttot * P], BF16)
    catL = nc.dram_tensor("catL", [c.R, CA], BF16)
    njloc = nc.dram_tensor("njloc", [c.R, NJ], BF16)
    catGa = nc.dram_tensor("catGa", [c.npad, CA], BF16, addr_space="Shared")
    catGb = nc.dram_tensor("catGb", [c.npad, CA], BF16, addr_space="Shared")

    rg = [list(range(c.ndev))]

    chunks = []
    cc0 = 0
    while cc0 < T:
        chunks.append((cc0, min(G, T - cc0)))
        cc0 += G

    with tile.TileContext(nc) as tc:
        with tc.tile_pool(name="persist", bufs=1) as pp:
            iota_i = pp.tile([P, P], I32, tag="iota_i")
            nc.gpsimd.iota(iota_i[:], pattern=[[1, P]], base=0, channel_multiplier=0)
            iota_b = pp.tile([P, P], BF16, tag="iota_b")
            nc.vector.tensor_copy(out=iota_b[:], in_=iota_i[:])
            ident = pp.tile([P, P], BF16, tag="ident")
            iota_c = pp.tile([P, 1], I32, tag="iota_c")
            nc.gpsimd.iota(iota_c[:], pattern=[[1, 1]], base=0, channel_multiplier=1)
            iota_cb = pp.tile([P, 1], BF16, tag="iota_cb")
            nc.vector.tensor_copy(out=iota_cb[:], in_=iota_c[:])
            nc.vector.tensor_tensor(
                out=ident[:],
                in0=iota_b[:],
                in1=bass.AP(iota_cb[:].tensor, iota_cb[:].offset,
                            [iota_cb[:].ap[0], [0, P]]),
                op=ALU.is_equal,
            )

            ilo = pp.tile([P, c.nblk * T1 * 8], I16, tag="ilo")
            ihi = pp.tile([P, c.nblk * T2 * 8], I16, tag="ihi")
            idst = pp.tile([P, c.nblk * T * 8], I16, tag="idst")
            dstloc = pp.tile([P, Ttot], BF16, tag="dstloc")
            nc.sync.dma_start(out=ilo[:], in_=pr["idx_lo"][:])
            nc.sync.dma_start(out=ihi[:], in_=pr["idx_hi"][:])
            nc.sync.dma_start(out=idst[:], in_=pr["idx_dst"][:])
            nc.sync.dma_start(out=dstloc[:], in_=pr["dstloc"][:])

            wfij_sb, wcat_sb, wnj_sb, attn_sb = [], [], [], []
            for l in range(3):
                cdim = c.odf if l == 0 else FD
                wf = pp.tile([cdim, FD], BF16, tag=f"wfij{l}")
                nc.sync.dma_start(out=wf[:], in_=pr[f"wfij{l}"][:])
                wfij_sb.append(wf)
                wc = pp.tile([FD + 1, CA], BF16, tag=f"wcat{l}")
                nc.sync.dma_start(out=wc[:], in_=pr[f"wcat{l}"][:])
                wcat_sb.append(wc)
                wn = pp.tile([FD + 1, NJ], BF16, tag=f"wnj{l}")
                nc.sync.dma_start(out=wn[:], in_=pr[f"wnj{l}"][:])
                wnj_sb.append(wn)
                ab = pp.tile([P, FD], BF16, tag=f"attn{l}")
                nc.sync.dma_start(out=ab[:], in_=pr[f"attn{l}"][:])
                attn_sb.append(ab)
            e3_sb = pp.tile([H, FD], F32, tag="e3")
            nc.sync.dma_start(out=e3_sb[:], in_=pr["e3"][:])

            nh_slice = pp.tile([FD + 1, c.R], BF16, tag="nh_slice")

            def tables_block(qC, wC, b, wc, wn):
                ct = qC.tile([P, CA], F32, tag="ct")
                nc.tensor.matmul(
                    out=ct[:],
                    lhsT=nh_slice[:, b * P : (b + 1) * P],
                    rhs=wc[:],
                    start=True,
                    stop=True,
                )
                cs = wC.tile([P, CA], BF16, tag="cs")
                nc.scalar.activation(out=cs[:], in_=ct[:], func=AF.Copy)
                nc.sync.dma_start(out=catL[b * P : (b + 1) * P, :], in_=cs[:])
                ctn = qC.tile([P, NJ], F32, tag="ctn")
                nc.tensor.matmul(
                    out=ctn[:],
                    lhsT=nh_slice[:, b * P : (b + 1) * P],
                    rhs=wn[:],
                    start=True,
                    stop=True,
                )
                cn = wC.tile([P, NJ], BF16, tag="cn")
                nc.scalar.activation(out=cn[:], in_=ctn[:], func=AF.Copy)
                nc.sync.dma_start(out=njloc[b * P : (b + 1) * P, :], in_=cn[:])

            for rep in range(reps):
              # ---- layer 0 tables prologue ----
              nc.sync.dma_start(out=nh_slice[:], in_=pr["nh0T"][:])
              with (
                tc.tile_pool(name=f"t0_{rep}", bufs=2) as wC0,
                tc.tile_pool(name=f"t0p_{rep}", bufs=2, space="PSUM") as qC0,
              ):
                for b in range(c.nblk):
                    tables_block(qC0, wC0, b, wcat_sb[0], wnj_sb[0])
              if "coll" not in ablate:
                  nc.gpsimd.collective_compute(
                      "AllGather", ALU.bypass, replica_groups=rg,
                      ins=[catL[:]], outs=[catGa[:]],
                  )

              for l in range(3):
                cdim = c.odf if l == 0 else FD
                catG = catGa if l % 2 == 0 else catGb
                catG_next = catGb if l % 2 == 0 else catGa
                f_src = None if l == 0 else (f1 if l == 1 else f2)
                f_dst = f1 if l == 0 else (f2 if l == 1 else None)
                store_f = l < 2
                wf = wfij_sb[l]
                ab = attn_sb[l]
                catLoV = bass.AP(catG[:].tensor, 0, [[CA, NLO], [1, CA]])
                if c.npad > LOHI:
                    catHiV = bass.AP(catG[:].tensor, LOHI * CA,
                                     [[CA, NHI], [1, CA]])
                else:
                    catHiV = catLoV

                if debug and l == 0:
                    nc.sync.dma_start(out=dbg["d_cat"][:], in_=catGa[:])
                if debug and l == 1:
                    nc.sync.dma_start(out=dbg["d_nh"][:], in_=nh_slice[:])
                with (
                    tc.tile_pool(name=f"pA{l}_{rep}", bufs=2) as pA,
                    tc.tile_pool(name=f"pB{l}_{rep}", bufs=3) as pB,
                    tc.tile_pool(name=f"qF{l}_{rep}", bufs=2, space="PSUM") as qF,
                    tc.tile_pool(name=f"qH{l}_{rep}", bufs=2, space="PSUM") as qH,
                    tc.tile_pool(name=f"qC{l}_{rep}", bufs=1, space="PSUM") as qC,
                ):
                    qrr = [0]

                    def gather_rows(gtile, col0, in_ap, itile, icol0, ntiles,
                                    elem):
                        # SWDGE ring limit: <=1024 indices (8 tiles) per inst
                        done = 0
                        while done < ntiles:
                            k = min(8, ntiles - done)
                            nc.gpsimd.dma_gather(
                                out_ap=bass.AP(
                                    gtile[:].tensor,
                                    gtile[:].offset + (col0 + done) * elem,
                                    [gtile[:].ap[0], [elem, k], [1, elem]]),
                                in_ap=in_ap,
                                idxs_ap=itile[:, icol0 + done * 8
                                              : icol0 + (done + k) * 8],
                                num_idxs=k * P,
                                num_idxs_reg=k * P,
                                elem_size=elem,
                                queue_num=qrr[0] % nq,
                            )
                            qrr[0] += 1
                            done += k

                    for b in range(c.nblk):
                        t0 = b * T
                        # ---- block-level gathers ----
                        gatA = pA.tile([P, T * CA], BF16, tag="gatA")
                        gatB = pA.tile([P, T * NJ], BF16, tag="gatB")
                        if "gather" not in ablate:
                            gather_rows(gatA, 0, catLoV, ilo, b * T1 * 8, T1, CA)
                            gather_rows(gatA, T1, catHiV, ihi, b * T2 * 8, T2, CA)
                            gather_rows(gatB, 0, njloc[:, :], idst, b * T * 8, T, NJ)
                        else:
                            gather_rows(gatA, 0, catLoV, ilo, b * T1 * 8, 1, CA)
                            gather_rows(gatB, 0, njloc[:, :], idst, b * T * 8, 1, NJ)
                        if l == 0:
                            efc = pA.tile([c.odf, T * P], BF16, tag="efc")
                            if "efc" not in ablate:
                                nc.sync.dma_start(
                                    out=efc[:],
                                    in_=pr["ef0T"][:, t0 * P : (t0 + T) * P],
                                )
                            else:
                                nc.sync.dma_start(
                                    out=efc[:, 0:P],
                                    in_=pr["ef0T"][:, t0 * P : (t0 + 1) * P],
                                )
                        elif "efc" in ablate:
                            efc = pA.tile([P, T * P], BF16, tag="efc")
                            nc.sync.dma_start_transpose(
                                out=efc[:].rearrange("p (t q) -> p t q", t=T)[:, 0:1, :],
                                in_=bass.AP(
                                    f_src[:].tensor, t0 * P,
                                    [f_src[:].ap[0], [1, P]],
                                ),
                            )
                        else:
                            efc = pA.tile([P, T * P], BF16, tag="efc")
                            nc.sync.dma_start_transpose(
                                out=efc[:].rearrange("p (t q) -> p t q", t=T),
                                in_=bass.AP(
                                    f_src[:].tensor, t0 * P,
                                    [f_src[:].ap[0], [1, T * P]],
                                ),
                            )
                        # one-hot [p_edge, (t, v)]
                        ohE = pA.tile([P, T * P], BF16, tag="ohE")
                        iov = bass.AP(iota_b[:].tensor, iota_b[:].offset,
                                      [iota_b[:].ap[0], [0, T], [1, P]])
                        dlv = bass.AP(dstloc[:].tensor, dstloc[:].offset + t0,
                                      [dstloc[:].ap[0], [1, T], [0, P]])
                        nc.vector.tensor_tensor(
                            out=ohE[:].rearrange("p (t v) -> p t v", t=T),
                            in0=iov, in1=dlv, op=ALU.is_equal,
                        )
                        e_sb = pA.tile([P, T * H], F32, tag="e_sb")
                        msg = pA.tile([P, T * (FD + H)], BF16, tag="msg")
                        if store_f:
                            frB = pA.tile([P, T * FD], BF16, tag="frB")
                        hT = qH.tile([FD + H, P], F32, tag="hT")

                        # ---- pass A: f_pre, leaky, logits ----
                        for (cc, g) in (chunks[:1] if "passA" in ablate else chunks):
                            W = g * FD
                            fp = qF.tile([P, G * FD], F32, tag="fp")
                            ni_v = bass.AP(
                                gatA[:].tensor, gatA[:].offset + cc * CA,
                                [gatA[:].ap[0], [CA, g], [1, FD]],
                            )
                            nc.tensor.matmul(
                                out=fp[:, 0:W], lhsT=ident[:], rhs=ni_v,
                                start=True, stop=False, skip_group_check=True,
                            )
                            nj_v = bass.AP(
                                gatB[:].tensor, gatB[:].offset + cc * NJ,
                                [gatB[:].ap[0], [NJ, g], [1, FD]],
                            )
                            nc.tensor.matmul(
                                out=fp[:, 0:W], lhsT=ident[:], rhs=nj_v,
                                start=False, stop=False, skip_group_check=True,
                            )
                            for j in range(g):
                                nc.tensor.matmul(
                                    out=fp[:, j * FD : (j + 1) * FD],
                                    lhsT=efc[0:cdim, (cc + j) * P : (cc + j + 1) * P],
                                    rhs=wf[:],
                                    start=False, stop=True,
                                    skip_group_check=True,
                                )
                            if store_f:
                                nc.scalar.activation(
                                    out=frB[:, cc * FD : (cc + g) * FD],
                                    in_=fp[:, 0:W], func=AF.Relu,
                                )
                            # leaky(x) = 0.01*x + 0.99*relu(x)
                            a1 = pB.tile([P, G * FD], BF16, tag="a1")
                            nc.scalar.activation(
                                out=a1[:, 0:W], in_=fp[:, 0:W],
                                func=AF.Relu, scale=0.99,
                            )
                            fl = pB.tile([P, G * FD], BF16, tag="fl")
                            nc.vector.scalar_tensor_tensor(
                                out=fl[:, 0:W], in0=fp[:, 0:W], scalar=0.01,
                                in1=a1[:, 0:W], op0=ALU.mult, op1=ALU.add,
                            )
                            if debug and l == 0 and b == 0 and cc == 0:
                                dfp = pB.tile([P, G * FD], F32, tag="dfp")
                                nc.scalar.activation(
                                    out=dfp[:], in_=fp[:], func=AF.Copy)
                                nc.sync.dma_start(out=dbg["d_fp"][:], in_=dfp[:])
                            scrm = pB.tile([P, G * FD], BF16, tag="scrm")
                            fl_v = bass.AP(
                                fl[:].tensor, fl[:].offset,
                                [fl[:].ap[0], [FD, g], [HE, H], [1, HE]],
                            )
                            a_v = bass.AP(
                                ab[:].tensor, ab[:].offset,
                                [ab[:].ap[0], [0, g], [HE, H], [1, HE]],
                            )
                            nc.vector.tensor_tensor(
                                out=bass.AP(
                                    scrm[:].tensor, scrm[:].offset,
                                    [scrm[:].ap[0], [FD, g], [HE, H], [1, HE]],
                                ),
                                in0=fl_v, in1=a_v, op=ALU.mult,
                            )
                            nc.vector.tensor_reduce(
                                out=bass.AP(
                                    e_sb[:].tensor, e_sb[:].offset + cc * H,
                                    [e_sb[:].ap[0], [H, g], [1, H]],
                                ),
                                in_=bass.AP(
                                    scrm[:].tensor, scrm[:].offset,
                                    [scrm[:].ap[0], [FD, g], [HE, H], [1, HE]],
                                ),
                                axis=mybir.AxisListType.X, op=ALU.add,
                            )
                        if store_f:
                            fd_v = bass.AP(
                                f_dst[:].tensor, t0 * P,
                                [f_dst[:].ap[0], [P, T], [1, FD]],
                            )
                            nc.sync.dma_start(out=fd_v, in_=frB[:])

                        # ---- softmax pieces ----
                        nc.vector.tensor_scalar(
                            out=e_sb[:], in0=e_sb[:], scalar1=60.0,
                            scalar2=None, op0=ALU.min,
                        )
                        nc.scalar.activation(
                            out=bass.AP(
                                msg[:].tensor, msg[:].offset + FD,
                                [msg[:].ap[0], [FD + H, T], [1, H]],
                            ),
                            in_=e_sb[:].rearrange("p (t h) -> p t h", t=T),
                            func=AF.Exp,
                        )
                        ns_v = bass.AP(
                            gatA[:].tensor, gatA[:].offset + FD,
                            [gatA[:].ap[0], [CA, T], [HE, H], [1, HE]],
                        )
                        ex_v = bass.AP(
                            msg[:].tensor, msg[:].offset + FD,
                            [msg[:].ap[0], [FD + H, T], [1, H], [0, HE]],
                        )
                        nc.vector.tensor_tensor(
                            out=bass.AP(
                                msg[:].tensor, msg[:].offset,
                                [msg[:].ap[0], [FD + H, T], [HE, H], [1, HE]],
                            ),
                            in0=ns_v, in1=ex_v, op=ALU.mult,
                        )
                        if debug and l == 0 and b == 0:
                            nc.sync.dma_start(out=dbg["d_e"][:], in_=e_sb[:])
                            nc.sync.dma_start(out=dbg["d_msg"][:], in_=msg[:])
                            nc.sync.dma_start(out=dbg["d_oh"][:], in_=ohE[:])
                            nc.sync.dma_start(out=dbg["d_gatA"][:], in_=gatA[:])
                            nc.sync.dma_start(out=dbg["d_gatB"][:], in_=gatB[:])
                        # ---- scatter ----
                        for t in range(T if "scatter" not in ablate else 1):
                            nc.tensor.matmul(
                                out=hT[:],
                                lhsT=msg[:, t * (FD + H) : (t + 1) * (FD + H)],
                                rhs=ohE[:, t * P : (t + 1) * P],
                                start=(t == 0),
                                stop=(t == (T - 1 if "scatter" not in ablate else 0)),
                                skip_group_check=True,
                            )
                        if debug and l == 0 and b == 0:
                            dhT = pB.tile([FD + H, P], F32, tag="dhT")
                            nc.scalar.activation(
                                out=dhT[:], in_=hT[:], func=AF.Copy)
                            nc.sync.dma_start(out=dbg["d_hT"][:], in_=dhT[:])
                        # ---- finalize ----
                        srec = pB.tile([H, P], F32, tag="srec")
                        nc.vector.tensor_scalar_add(
                            out=srec[:], in0=hT[FD : FD + H, :], scalar1=EPS,
                        )
                        rcp = pB.tile([H, P], F32, tag="rcp")
                        nc.vector.reciprocal(out=rcp[:], in_=srec[:])
                        rcp96 = qC.tile([FD, P], F32, tag="rcp96")
                        nc.tensor.matmul(
                            out=rcp96[:], lhsT=e3_sb[:], rhs=rcp[:],
                            start=True, stop=True,
                        )
                        rcp96s = pB.tile([FD, P], F32, tag="rcp96s")
                        nc.scalar.activation(
                            out=rcp96s[:], in_=rcp96[:], func=AF.Copy,
                        )
                        if l < 2:
                            nc.vector.scalar_tensor_tensor(
                                out=nh_slice[0:FD, b * P : (b + 1) * P],
                                in0=hT[0:FD, :], scalar=0.0,
                                in1=rcp96s[:], op0=ALU.max, op1=ALU.mult,
                            )
                            tables_block(qC, pB, b, wcat_sb[l + 1], wnj_sb[l + 1])
                        else:
                            hfin = pB.tile([FD, P], F32, tag="hfin")
                            nc.vector.scalar_tensor_tensor(
                                out=hfin[:], in0=hT[0:FD, :], scalar=1.0,
                                in1=rcp96s[:], op0=ALU.mult, op1=ALU.mult,
                            )
                            nc.sync.dma_start(
                                out=out3T[:, b * P : (b + 1) * P], in_=hfin[:],
                            )

                if l < 2 and "coll" not in ablate:
                    nc.gpsimd.collective_compute(
                        "AllGather", ALU.bypass, replica_groups=rg,
                        ins=[catL[:]], outs=[catG_next[:]],
                    )

    nc.compile()
    return nc


_CACHE = {}


def get_program(cfg, reps=1):
    k = cfg.key + (reps,)
    if k not in _CACHE:
        _CACHE[k] = build_program(cfg, reps=reps)
    return _CACHE[k]


def run(inputs, cfg, core_ids=None, trace=False):
    from concourse.bass_utils import run_bass_kernel_spmd

    percore = host_prep(inputs, cfg)
    nc = get_program(cfg)
    if core_ids is None:
        core_ids = list(range(cfg.ndev))
    res = run_bass_kernel_spmd(nc, percore, core_ids, trace=trace)
    outs = [res.results[i]["out3T"].T for i in range(cfg.ndev)]
    full = np.concatenate(outs, axis=0)  # [npad, 96]
    return full, res


def make_cfg(inputs):
    src = np.asarray(inputs["src"]).astype(np.int64)
    dst = np.asarray(inputs["dst"]).astype(np.int64)
    n_real = 50000
    ndev = 8
    nblk = 49
    npad = ndev * nblk * P
    nb_all = npad // P
    blk = dst // P
    lo = src < LOHI
    cnt_lo = np.bincount(blk[lo], minlength=nb_all)
    cnt_hi = np.bincount(blk[~lo], minlength=nb_all)
    T1 = max(1, int(np.ceil(cnt_lo.max() / P)))
    T2 = max(1, int(np.ceil(cnt_hi.max() / P)))
    odf = np.asarray(inputs["countyodfeats"]).shape[1]
    return Cfg(ndev, nblk, T1, T2, odf, n_real)


def kernel(**inputs) -> np.ndarray:
    cfg = make_cfg(inputs)
    full, _ = run(inputs, cfg)
    idxs = np.asarray(inputs["idxs"]).astype(np.int64)
    return np.ascontiguousarray(full[idxs]).astype(np.float32)
